# revision 43
# baseline (speedup 1.0000x reference)
"""MixedArityTreeLSTM Trainium2 kernel (v2).

Level-synchronous bottom-up Tree-LSTM over B=256 heap-indexed perfect binary
trees (511 nodes, depth 8), E=H=128. Pure data-parallel over 8 NeuronCores
(32 trees per core); weights replicated.

v2 redesign vs v1:
- Embedding gather done on HOST (numpy take); x streamed in as plain DMA.
- Per-level "split" column order: level l+1 is stored [left-children |
  right-children] of level l's column order, so every child read (h_e, h_o,
  c_e, c_o) is a contiguous slice and DVE mask-multiplies run in fast mode.
- Ubt' = Ubt - Uun folding: pre = W x + Ubt'·(m h_l) + Ubb·(m h_r) + Uun·h_l,
  so no (1-m) mask op is needed.
- Per-gate bias+mask-delta applied with one K=2 matmul ([b_g; db_g]^T
  [ones; m]) so gate activations need no ACT bias -> 4 sigmoid gates are
  activated in ONE batched ACT instruction over a packed PSUM tile.
- fr gate's unary kill: multiply c_o by the arity mask (Pool engine) instead
  of a +-40 bias hack.
- Elementwise c/h chain split across DVE and Pool engines.
"""

import numpy as np
import ml_dtypes

B, D = 256, 8
V, E, H = 32000, 128, 128
NCORES = 8
BL = B // NCORES  # 32 trees per core

LVLN = {l: BL * (2 ** l) for l in range(D + 1)}  # cols per level per core
N_INT = sum(LVLN[l] for l in range(D))  # 8160 internal cols
N_ALL = N_INT + LVLN[D]  # 16352

# x layout: leaf level first, then levels 7..0
XOFF = {}
_o = 0
for l in [D] + list(range(D - 1, -1, -1)):
    XOFF[l] = _o
    _o += LVLN[l]
# mask layout: levels 7..0
MOFF = {}
_o = 0
for l in range(D - 1, -1, -1):
    MOFF[l] = _o
    _o += LVLN[l]

WC = 256          # internal chunk width
WC_LEAF = 512     # leaf chunk width
# cascade chunk widths for levels 6..0 (halving down the tree)
WCL = {6: 256, 5: 256, 4: 128, 3: 64, 2: 64, 1: 32, 0: 32}

BF16 = ml_dtypes.bfloat16

# split-order permutations: perm[l][p] = flat tree-major index (t*2^l + j)
PERM = {0: np.arange(BL, dtype=np.int64) * 1}
for l in range(D):
    e = PERM[l]
    t, j = e >> l, e & ((1 << l) - 1)
    left = (t << (l + 1)) + 2 * j
    PERM[l + 1] = np.concatenate([left, left + 1])

_CACHE = {}


def _build_nc():
    if "nc" in _CACHE:
        return _CACHE["nc"]

    from contextlib import ExitStack

    import concourse.mybir as mybir
    import concourse.tile as tile
    from concourse import bacc

    dt = mybir.dt
    AF = mybir.ActivationFunctionType

    nc = bacc.Bacc()

    xall_d = nc.dram_tensor("xall", [128, N_ALL], dt.bfloat16, kind="ExternalInput")
    mbc_d = nc.dram_tensor("mbc", [128, N_INT], dt.bfloat16, kind="ExternalInput")
    onesm_d = nc.dram_tensor("onesm", [2, N_INT], dt.bfloat16, kind="ExternalInput")
    wq_d = nc.dram_tensor("wq", [E, 4, H], dt.bfloat16, kind="ExternalInput")
    ubtp_d = nc.dram_tensor("ubtp", [H, 5, H], dt.bfloat16, kind="ExternalInput")
    ubb_d = nc.dram_tensor("ubb", [H, 5, H], dt.bfloat16, kind="ExternalInput")
    uun_d = nc.dram_tensor("uun", [H, 4, H], dt.bfloat16, kind="ExternalInput")
    bd_d = nc.dram_tensor("bd", [2, 5, H], dt.bfloat16, kind="ExternalInput")
    bleaf_d = nc.dram_tensor("bleaf", [H, 1], dt.float32, kind="ExternalInput")

    h_out_d = nc.dram_tensor("h_out", [H, BL], dt.float32, kind="ExternalOutput")
    c_out_d = nc.dram_tensor("c_out", [H, BL], dt.float32, kind="ExternalOutput")

    with tile.TileContext(nc) as tc, ExitStack() as ctx:
        consts = ctx.enter_context(tc.tile_pool(name="consts", bufs=1))

        wq = consts.tile([E, 4, H], dt.bfloat16)
        ubtp = consts.tile([H, 5, H], dt.bfloat16)
        ubb = consts.tile([H, 5, H], dt.bfloat16)
        uun = consts.tile([H, 4, H], dt.bfloat16)
        bd = consts.tile([2, 5, H], dt.bfloat16)
        bleaf = consts.tile([H, 1], dt.float32)

        xall = consts.tile([128, N_ALL], dt.bfloat16, name="xall", tag="xall")
        mbc = consts.tile([128, N_INT], dt.bfloat16, name="mbc", tag="mbc")
        onesm = consts.tile([2, N_INT], dt.bfloat16)

        def dx(eng, t, d, a, b):
            eng.dma_start(out=t[:, a:b], in_=d[:, a:b])

        # All DMA issue work on SP + Pool so the ACT engine stays free for
        # activations. Leaf x + W first so PE can start; L7 masks + x next.
        nc.scalar.dma_start(out=bleaf, in_=bleaf_d[:, :])
        dx(nc.sync, xall, xall_d, 0, 512)
        nc.sync.dma_start(out=wq, in_=wq_d[:, :, :])
        dx(nc.sync, xall, xall_d, 512, 2048)
        dx(nc.gpsimd, xall, xall_d, 4096, 6144)
        dx(nc.sync, mbc, mbc_d, 0, 1024)
        dx(nc.sync, onesm, onesm_d, 0, 1024)
        nc.sync.dma_start(out=ubtp, in_=ubtp_d[:, :, :])
        dx(nc.gpsimd, xall, xall_d, 8192, 12288)
        dx(nc.sync, xall, xall_d, 2048, 4096)
        nc.sync.dma_start(out=ubb, in_=ubb_d[:, :, :])
        nc.sync.dma_start(out=uun, in_=uun_d[:, :, :])
        nc.sync.dma_start(out=bd, in_=bd_d[:, :, :])
        dx(nc.gpsimd, xall, xall_d, 6144, 8192)
        dx(nc.sync, mbc, mbc_d, 1024, 2048)
        dx(nc.sync, onesm, onesm_d, 1024, 2048)
        dx(nc.sync, mbc, mbc_d, 2048, 4096)
        dx(nc.sync, onesm, onesm_d, 2048, 4096)
        dx(nc.sync, mbc, mbc_d, 4096, N_INT)
        dx(nc.sync, onesm, onesm_d, 4096, N_INT)
        dx(nc.sync, xall, xall_d, 12288, N_ALL)

        lev = ctx.enter_context(tc.tile_pool(name="lev", bufs=1))
        h_t, c_t = {}, {}
        h_t[D] = lev.tile([H, LVLN[D]], dt.bfloat16, name="h8", tag="h8")
        for l in range(D - 1, 0, -1):
            h_t[l] = lev.tile([H, LVLN[l]], dt.bfloat16, name=f"h{l}", tag=f"h{l}")
            c_t[l] = lev.tile([H, LVLN[l]], dt.float32, name=f"c{l}", tag=f"c{l}")
        h_t[0] = lev.tile([H, BL], dt.float32, name="h0", tag="h0")
        c_t[0] = lev.tile([H, BL], dt.float32, name="c0", tag="c0")

        psL = ctx.enter_context(tc.tile_pool(name="psL", bufs=2, space="PSUM"))
        psG = ctx.enter_context(tc.tile_pool(name="psG", bufs=2, space="PSUM"))
        work = ctx.enter_context(tc.tile_pool(name="work", bufs=3))

        # gate specs: (region, w_idx, ubtp_idx, ubb_idx, uun_idx|None, bd_idx)
        G_FULL = [
            (0, 0, 0, 0, 0, 0),   # i
            (1, 1, 1, 1, 1, 1),   # f_l
            (2, 1, 2, 2, None, 2),  # f_r (no unary path; c_o masked instead)
            (3, 2, 3, 3, 2, 3),   # o
            (4, 3, 4, 4, 3, 4),   # u
        ]
        G_TOP = [
            (0, 0, 0, 0, 0, 0),   # i
            (1, 2, 3, 3, 2, 3),   # o
            (2, 3, 4, 4, 3, 4),   # u
        ]

        import os as _os
        WARM_N = int(_os.environ.get("TL_WARM", "12"))

        def emit_leaf(k):
            """Leaf chunk k: h = tanh(W3^T x + b3) over cols [512k, 512k+512)."""
            s = slice(k * WC_LEAF, (k + 1) * WC_LEAF)
            ps = psL.tile([H, WC_LEAF], dt.float32, tag="psl", name="psl")
            nc.tensor.matmul(ps, wq[:, 3, :], xall[:, s], start=True, stop=True)
            nc.scalar.activation(h_t[D][:, s], ps, AF.Tanh, bias=bleaf[:, 0:1])

        def emit_masks(l, c0, wc):
            """Mask-multiplied operands for one chunk (DVE/Pool work that can
            run a chunk ahead of the matmuls)."""
            N = LVLN[l]
            top = l == D - 1
            hch, cch = h_t[l + 1], (None if top else c_t[l + 1])
            ms = slice(MOFF[l] + c0, MOFF[l] + c0 + wc)
            le = slice(c0, c0 + wc)
            ro = slice(N + c0, N + c0 + wc)

            hm = work.tile([128, 1024], dt.bfloat16, tag="hm", name="hm")
            hm = hm.rearrange("p (g n) -> p g n", n=wc)
            nc.vector.tensor_mul(hm[:, 0, :], hch[:, le], mbc[:, ms])
            nc.vector.tensor_mul(hm[:, 1, :], hch[:, ro], mbc[:, ms])
            com = None
            if not top:
                com = work.tile([128, 512], dt.float32, tag="com", name="com")
                com = com[:, 0:wc]
                nc.gpsimd.tensor_mul(com, cch[:, ro], mbc[:, ms])
            return hm, com

        def emit_chunk(l, c0, wc, pre=None):
            N = LVLN[l]
            top = l == D - 1
            gates = G_TOP if top else G_FULL
            nsig = 2 if top else 4
            iu_, io_, iuu_ = (0, 1, 2) if top else (0, 3, 4)
            hch, cch = h_t[l + 1], (None if top else c_t[l + 1])

            xs = slice(XOFF[l] + c0, XOFF[l] + c0 + wc)
            ms = slice(MOFF[l] + c0, MOFF[l] + c0 + wc)
            le = slice(c0, c0 + wc)            # left child cols
            ro = slice(N + c0, N + c0 + wc)    # right child cols
            ls = slice(c0, c0 + wc)            # this level's cols

            hm, com = pre if pre is not None else emit_masks(l, c0, wc)

            # flat 3-bank PSUM tile viewed as ngate x wc regions
            psf = psG.tile([H, 1536], dt.float32, tag="psg", name="psg")
            ps = psf.rearrange("p (g n) -> p g n", n=wc)
            # When gate regions are bank-aligned (wc=512), issue all x-only
            # matmuls first: they have no h dependency, so the PE can chew
            # them while the previous level's activation tail drains.
            # (Non-bank-aligned regions share a PSUM zero region, which
            # forbids concurrently open accumulation groups.)
            wfirst = (wc * 4) % 2048 == 0
            if wfirst:
                for r, wi, ti, bi, ui, di in gates:
                    nc.tensor.matmul(ps[:, r, :], wq[:, wi, :], xall[:, xs],
                                     start=True, stop=False)
            for r, wi, ti, bi, ui, di in gates:
                po = ps[:, r, :]
                if not wfirst:
                    nc.tensor.matmul(po, wq[:, wi, :], xall[:, xs],
                                     start=True, stop=False)
                nc.tensor.matmul(po, ubtp[:, ti, :], hm[:, 0, :],
                                 start=False, stop=False)
                nc.tensor.matmul(po, ubb[:, bi, :], hm[:, 1, :],
                                 start=False, stop=False)
                if ui is not None:
                    nc.tensor.matmul(po, uun[:, ui, :], hch[:, le],
                                     start=False, stop=False)
                nc.tensor.matmul(
                    po, bd[:, di, :], onesm[:, ms], start=False, stop=True
                )

            gs = work.tile([128, 2560], dt.float32, tag="gs", name="gs")
            gs = gs.rearrange("p (g n) -> p g n", n=wc)
            nc.scalar.activation(gs[:, 0:nsig, :], ps[:, 0:nsig, :], AF.Sigmoid)
            nc.scalar.activation(gs[:, iuu_, :], ps[:, iuu_, :], AF.Tanh)

            cs = c_t[l][:, ls]
            if top:
                nc.vector.tensor_mul(cs, gs[:, iu_, :], gs[:, iuu_, :])
            else:
                t1 = work.tile([128, 512], dt.float32, tag="t1", name="t1")[:, 0:wc]
                nc.vector.tensor_mul(t1, gs[:, 0, :], gs[:, 4, :])
                t2 = work.tile([128, 512], dt.float32, tag="t2", name="t2")[:, 0:wc]
                nc.gpsimd.tensor_mul(t2, gs[:, 1, :], cch[:, le])
                t3 = work.tile([128, 512], dt.float32, tag="t3", name="t3")[:, 0:wc]
                nc.vector.tensor_mul(t3, gs[:, 2, :], com)
                a1 = work.tile([128, 512], dt.float32, tag="a1", name="a1")[:, 0:wc]
                nc.gpsimd.tensor_add(a1, t1, t2)
                nc.vector.tensor_add(cs, a1, t3)

            tch = work.tile([128, 512], dt.float32, tag="tch", name="tch")[:, 0:wc]
            nc.scalar.activation(tch, cs, AF.Tanh)
            nc.vector.tensor_mul(h_t[l][:, ls], gs[:, io_, :], tch)

        def emit_warm():
            """Keep-warm filler so the PE p-state clock stays hot across the
            serial dependency tail of the small levels. Writes rotate over
            disjoint PSUM slices so the dummies pipeline with no WAW stalls."""
            wps = psL.tile([H, WC_LEAF], dt.float32, tag="psl", name="warm")
            for i in range(WARM_N):
                o = (i % 4) * 128
                nc.tensor.matmul(
                    wps[:, o : o + 128], wq[:, 0, :], xall[:, 512 : 512 + 128],
                    start=True, stop=True, skip_group_check=True,
                )

        # emission plan: leaf + L7 software-pipelined in pair order (L7
        # chunk k needs the leaf pair k), then the cascade: chunk sizes
        # halve down the tree and go in (j, Q/2+j) pair order so parent
        # chunks become ready while the child level is still draining.
        items = []
        for k in range(8):
            items += [("leaf", k), ("leaf", 8 + k), ("chunk", D - 1, 512 * k, 512)]
        for l in range(D - 2, -1, -1):
            if l <= 5 and WARM_N:
                items.append(("warm",))
            wc = min(WC, LVLN[l])
            Q = max(1, LVLN[l] // wc)
            order = [q for p in range(Q // 2) for q in (p, Q // 2 + p)] or [0]
            items += [("chunk", l, j * wc, wc) for j in order]

        # pipelined emission: each chunk's mask ops go on the DVE/Pool
        # queues one chunk BEFORE the previous chunk's compute tail, so the
        # PE's operands are ready at chunk boundaries. (Only for l >= 4 —
        # deeper levels' masks would order before a same-queue h write they
        # depend on.)
        prev = None
        for it in items:
            if it[0] == "chunk" and it[1] >= 4:
                _, l_, c0_, wc_ = it
                pre_ = emit_masks(l_, c0_, wc_)
                if prev:
                    prev()
                prev = (lambda l_=l_, c0_=c0_, wc_=wc_, pre_=pre_:
                        emit_chunk(l_, c0_, wc_, pre_))
            else:
                if prev:
                    prev()
                    prev = None
                if it[0] == "leaf":
                    emit_leaf(it[1])
                elif it[0] == "warm":
                    emit_warm()
                else:
                    emit_chunk(it[1], it[2], it[3])
        if prev:
            prev()

        nc.sync.dma_start(out=h_out_d[:, :], in_=h_t[0][:, :])
        nc.sync.dma_start(out=c_out_d[:, :], in_=c_t[0][:, :])

    nc.finalize()
    _CACHE["nc"] = nc
    return nc


def prep_shared_inputs(emb, W, bW, Ubin, bUbin, Uun, bUun):
    emb = np.asarray(emb, np.float32)
    W = np.asarray(W, np.float32)
    bW = np.asarray(bW, np.float32)
    Ubin = np.asarray(Ubin, np.float32)
    bUbin = np.asarray(bUbin, np.float32)
    Uun = np.asarray(Uun, np.float32)
    bUun = np.asarray(bUun, np.float32)

    ubt = Ubin[:, :H, :]  # [5, H, H] top half (left child)
    ubb_ = Ubin[:, H:, :]  # bottom half (right child)
    # Ubt' = Ubt - Uun for gates with a unary path (i, fl, o, u)
    ubtp = ubt.copy()
    for gi_, ui_ in ((0, 0), (1, 1), (3, 2), (4, 3)):
        ubtp[gi_] = ubt[gi_] - Uun[ui_]

    # bias rows [b_g; db_g] per gate (i, fl, fr, o, u)
    bcom = np.stack([
        bW[0] + bUun[0],
        bW[1] + bUun[1],
        bW[1] + bUbin[2],
        bW[2] + bUun[2],
        bW[3] + bUun[3],
    ])
    bdel = np.stack([
        bUbin[0] - bUun[0],
        bUbin[1] - bUun[1],
        np.zeros(H, np.float32),
        bUbin[3] - bUun[2],
        bUbin[4] - bUun[3],
    ])
    bd = np.stack([bcom, bdel]).astype(BF16)  # [2, 5, H]

    return dict(
        emb_bf=emb.astype(BF16),
        wq=np.ascontiguousarray(W.transpose(1, 0, 2)).astype(BF16),
        ubtp=np.ascontiguousarray(ubtp.transpose(1, 0, 2)).astype(BF16),
        ubb=np.ascontiguousarray(ubb_.transpose(1, 0, 2)).astype(BF16),
        uun=np.ascontiguousarray(Uun.transpose(1, 0, 2)).astype(BF16),
        bd=bd,
        bleaf=bW[3].reshape(H, 1).astype(np.float32),
    )


def prep_core_inputs(tokens_c, arity_c, shared):
    """Per-core inputs. tokens_c [BL,511], arity_c [BL,255]."""
    tokens_c = np.asarray(tokens_c)
    arity_c = np.asarray(arity_c)
    emb_bf = shared["emb_bf"]

    xcols = np.empty((N_ALL, E), dtype=BF16)
    mrow = np.empty(N_INT, dtype=np.float32)
    for l in [D] + list(range(D - 1, -1, -1)):
        off, cnt = 2 ** l - 1, 2 ** l
        toks = tokens_c[:, off : off + cnt].reshape(-1)[PERM[l]]
        xcols[XOFF[l] : XOFF[l] + LVLN[l]] = emb_bf[toks]
        if l < D:
            ar = arity_c[:, off : off + cnt].reshape(-1)[PERM[l]]
            mrow[MOFF[l] : MOFF[l] + LVLN[l]] = (ar == 1).astype(np.float32)

    m16 = mrow.astype(BF16)
    onesm = np.stack([np.ones(N_INT, BF16), m16])  # [2, N_INT]
    out = {k: v for k, v in shared.items() if k != "emb_bf"}
    out["xall"] = np.ascontiguousarray(xcols.T)
    out["mbc"] = np.broadcast_to(m16, (128, N_INT)).copy()
    out["onesm"] = onesm
    return out


def kernel(tokens, arity, emb, W, bW, Ubin, bUbin, Uun, bUun):
    from concourse.bass_utils import run_bass_kernel_spmd

    tokens = np.asarray(tokens)
    arity = np.asarray(arity)

    shared = prep_shared_inputs(emb, W, bW, Ubin, bUbin, Uun, bUun)
    in_maps = [
        prep_core_inputs(
            tokens[k * BL : (k + 1) * BL], arity[k * BL : (k + 1) * BL], shared
        )
        for k in range(NCORES)
    ]

    nc = _build_nc()
    res = run_bass_kernel_spmd(nc, in_maps, core_ids=list(range(NCORES)))
    results = res.results

    h = np.concatenate([r["h_out"].T for r in results], axis=0)
    c = np.concatenate([r["c_out"].T for r in results], axis=0)
    return h.astype(np.float32), c.astype(np.float32)


# revision 44
# speedup vs baseline: 1.0384x; 1.0384x over previous
"""MixedArityTreeLSTM Trainium2 kernel (v2).

Level-synchronous bottom-up Tree-LSTM over B=256 heap-indexed perfect binary
trees (511 nodes, depth 8), E=H=128. Pure data-parallel over 8 NeuronCores
(32 trees per core); weights replicated.

v2 redesign vs v1:
- Embedding gather done on HOST (numpy take); x streamed in as plain DMA.
- Per-level "split" column order: level l+1 is stored [left-children |
  right-children] of level l's column order, so every child read (h_e, h_o,
  c_e, c_o) is a contiguous slice and DVE mask-multiplies run in fast mode.
- Ubt' = Ubt - Uun folding: pre = W x + Ubt'·(m h_l) + Ubb·(m h_r) + Uun·h_l,
  so no (1-m) mask op is needed.
- Per-gate bias+mask-delta applied with one K=2 matmul ([b_g; db_g]^T
  [ones; m]) so gate activations need no ACT bias -> 4 sigmoid gates are
  activated in ONE batched ACT instruction over a packed PSUM tile.
- fr gate's unary kill: multiply c_o by the arity mask (Pool engine) instead
  of a +-40 bias hack.
- Elementwise c/h chain split across DVE and Pool engines.
"""

import numpy as np
import ml_dtypes

B, D = 256, 8
V, E, H = 32000, 128, 128
NCORES = 8
BL = B // NCORES  # 32 trees per core

LVLN = {l: BL * (2 ** l) for l in range(D + 1)}  # cols per level per core
N_INT = sum(LVLN[l] for l in range(D))  # 8160 internal cols
N_ALL = N_INT + LVLN[D]  # 16352

# x layout: leaf level first, then levels 7..0
XOFF = {}
_o = 0
for l in [D] + list(range(D - 1, -1, -1)):
    XOFF[l] = _o
    _o += LVLN[l]
# mask layout: levels 7..0
MOFF = {}
_o = 0
for l in range(D - 1, -1, -1):
    MOFF[l] = _o
    _o += LVLN[l]

WC = 256          # internal chunk width
WC_LEAF = 512     # leaf chunk width
# cascade chunk widths for levels 6..0 (halving down the tree)
WCL = {6: 256, 5: 256, 4: 128, 3: 64, 2: 64, 1: 32, 0: 32}

BF16 = ml_dtypes.bfloat16

# split-order permutations: perm[l][p] = flat tree-major index (t*2^l + j)
PERM = {0: np.arange(BL, dtype=np.int64) * 1}
for l in range(D):
    e = PERM[l]
    t, j = e >> l, e & ((1 << l) - 1)
    left = (t << (l + 1)) + 2 * j
    PERM[l + 1] = np.concatenate([left, left + 1])

_CACHE = {}


def _build_nc():
    if "nc" in _CACHE:
        return _CACHE["nc"]

    from contextlib import ExitStack

    import concourse.mybir as mybir
    import concourse.tile as tile
    from concourse import bacc

    dt = mybir.dt
    AF = mybir.ActivationFunctionType

    nc = bacc.Bacc()

    xall_d = nc.dram_tensor("xall", [128, N_ALL], dt.bfloat16, kind="ExternalInput")
    mbc_d = nc.dram_tensor("mbc", [128, N_INT], dt.bfloat16, kind="ExternalInput")
    onesm_d = nc.dram_tensor("onesm", [2, N_INT], dt.bfloat16, kind="ExternalInput")
    wq_d = nc.dram_tensor("wq", [E, 4, H], dt.bfloat16, kind="ExternalInput")
    ubtp_d = nc.dram_tensor("ubtp", [H, 5, H], dt.bfloat16, kind="ExternalInput")
    ubb_d = nc.dram_tensor("ubb", [H, 5, H], dt.bfloat16, kind="ExternalInput")
    uun_d = nc.dram_tensor("uun", [H, 4, H], dt.bfloat16, kind="ExternalInput")
    bd_d = nc.dram_tensor("bd", [2, 5, H], dt.bfloat16, kind="ExternalInput")
    bleaf_d = nc.dram_tensor("bleaf", [H, 1], dt.float32, kind="ExternalInput")

    h_out_d = nc.dram_tensor("h_out", [H, BL], dt.float32, kind="ExternalOutput")
    c_out_d = nc.dram_tensor("c_out", [H, BL], dt.float32, kind="ExternalOutput")

    with tile.TileContext(nc) as tc, ExitStack() as ctx:
        consts = ctx.enter_context(tc.tile_pool(name="consts", bufs=1))

        wq = consts.tile([E, 4, H], dt.bfloat16)
        ubtp = consts.tile([H, 5, H], dt.bfloat16)
        ubb = consts.tile([H, 5, H], dt.bfloat16)
        uun = consts.tile([H, 4, H], dt.bfloat16)
        bd = consts.tile([2, 5, H], dt.bfloat16)
        bleaf = consts.tile([H, 1], dt.float32)

        xall = consts.tile([128, N_ALL], dt.bfloat16, name="xall", tag="xall")
        mbc = consts.tile([128, N_INT], dt.bfloat16, name="mbc", tag="mbc")
        onesm = consts.tile([2, N_INT], dt.bfloat16)

        def dx(eng, t, d, a, b):
            eng.dma_start(out=t[:, a:b], in_=d[:, a:b])

        # All DMA issue work on SP + Pool so the ACT engine stays free for
        # activations. Leaf x + W first so PE can start; L7 masks + x next.
        nc.scalar.dma_start(out=bleaf, in_=bleaf_d[:, :])
        dx(nc.sync, xall, xall_d, 0, 512)
        nc.sync.dma_start(out=wq, in_=wq_d[:, :, :])
        dx(nc.sync, xall, xall_d, 512, 2048)
        dx(nc.gpsimd, xall, xall_d, 4096, 6144)
        dx(nc.sync, mbc, mbc_d, 0, 1024)
        dx(nc.sync, onesm, onesm_d, 0, 1024)
        nc.sync.dma_start(out=ubtp, in_=ubtp_d[:, :, :])
        dx(nc.gpsimd, xall, xall_d, 8192, 12288)
        dx(nc.sync, xall, xall_d, 2048, 4096)
        nc.sync.dma_start(out=ubb, in_=ubb_d[:, :, :])
        nc.sync.dma_start(out=uun, in_=uun_d[:, :, :])
        nc.sync.dma_start(out=bd, in_=bd_d[:, :, :])
        dx(nc.gpsimd, xall, xall_d, 6144, 8192)
        dx(nc.sync, mbc, mbc_d, 1024, 2048)
        dx(nc.sync, onesm, onesm_d, 1024, 2048)
        dx(nc.sync, mbc, mbc_d, 2048, 4096)
        dx(nc.sync, onesm, onesm_d, 2048, 4096)
        dx(nc.sync, mbc, mbc_d, 4096, N_INT)
        dx(nc.sync, onesm, onesm_d, 4096, N_INT)
        dx(nc.sync, xall, xall_d, 12288, N_ALL)

        lev = ctx.enter_context(tc.tile_pool(name="lev", bufs=1))
        h_t, c_t = {}, {}
        h_t[D] = lev.tile([H, LVLN[D]], dt.bfloat16, name="h8", tag="h8")
        for l in range(D - 1, 0, -1):
            h_t[l] = lev.tile([H, LVLN[l]], dt.bfloat16, name=f"h{l}", tag=f"h{l}")
            c_t[l] = lev.tile([H, LVLN[l]], dt.float32, name=f"c{l}", tag=f"c{l}")
        h_t[0] = lev.tile([H, BL], dt.float32, name="h0", tag="h0")
        c_t[0] = lev.tile([H, BL], dt.float32, name="c0", tag="c0")

        psL = ctx.enter_context(tc.tile_pool(name="psL", bufs=2, space="PSUM"))
        psG = ctx.enter_context(tc.tile_pool(name="psG", bufs=2, space="PSUM"))
        work = ctx.enter_context(tc.tile_pool(name="work", bufs=3))

        # gate specs: (region, w_idx, ubtp_idx, ubb_idx, uun_idx|None, bd_idx)
        G_FULL = [
            (0, 0, 0, 0, 0, 0),   # i
            (1, 1, 1, 1, 1, 1),   # f_l
            (2, 1, 2, 2, None, 2),  # f_r (no unary path; c_o masked instead)
            (3, 2, 3, 3, 2, 3),   # o
            (4, 3, 4, 4, 3, 4),   # u
        ]
        G_TOP = [
            (0, 0, 0, 0, 0, 0),   # i
            (1, 2, 3, 3, 2, 3),   # o
            (2, 3, 4, 4, 3, 4),   # u
        ]

        import os as _os
        WARM_N = int(_os.environ.get("TL_WARM", "20"))

        def emit_leaf(k):
            """Leaf chunk k: h = tanh(W3^T x + b3) over cols [512k, 512k+512)."""
            s = slice(k * WC_LEAF, (k + 1) * WC_LEAF)
            ps = psL.tile([H, WC_LEAF], dt.float32, tag="psl", name="psl")
            nc.tensor.matmul(ps, wq[:, 3, :], xall[:, s], start=True, stop=True)
            nc.scalar.activation(h_t[D][:, s], ps, AF.Tanh, bias=bleaf[:, 0:1])

        def emit_masks(l, c0, wc):
            """Mask-multiplied operands for one chunk (DVE/Pool work that can
            run a chunk ahead of the matmuls)."""
            N = LVLN[l]
            top = l == D - 1
            hch, cch = h_t[l + 1], (None if top else c_t[l + 1])
            ms = slice(MOFF[l] + c0, MOFF[l] + c0 + wc)
            le = slice(c0, c0 + wc)
            ro = slice(N + c0, N + c0 + wc)

            hm = work.tile([128, 1024], dt.bfloat16, tag="hm", name="hm")
            hm = hm.rearrange("p (g n) -> p g n", n=wc)
            nc.vector.tensor_mul(hm[:, 0, :], hch[:, le], mbc[:, ms])
            nc.vector.tensor_mul(hm[:, 1, :], hch[:, ro], mbc[:, ms])
            com = None
            if not top:
                com = work.tile([128, 512], dt.float32, tag="com", name="com")
                com = com[:, 0:wc]
                nc.gpsimd.tensor_mul(com, cch[:, ro], mbc[:, ms])
            return hm, com

        def emit_chunk(l, c0, wc, pre=None):
            N = LVLN[l]
            top = l == D - 1
            gates = G_TOP if top else G_FULL
            nsig = 2 if top else 4
            iu_, io_, iuu_ = (0, 1, 2) if top else (0, 3, 4)
            hch, cch = h_t[l + 1], (None if top else c_t[l + 1])

            xs = slice(XOFF[l] + c0, XOFF[l] + c0 + wc)
            ms = slice(MOFF[l] + c0, MOFF[l] + c0 + wc)
            le = slice(c0, c0 + wc)            # left child cols
            ro = slice(N + c0, N + c0 + wc)    # right child cols
            ls = slice(c0, c0 + wc)            # this level's cols

            hm, com = pre if pre is not None else emit_masks(l, c0, wc)

            # flat 3-bank PSUM tile viewed as ngate x wc regions
            psf = psG.tile([H, 1536], dt.float32, tag="psg", name="psg")
            ps = psf.rearrange("p (g n) -> p g n", n=wc)
            # When gate regions are bank-aligned (wc=512), issue all x-only
            # matmuls first: they have no h dependency, so the PE can chew
            # them while the previous level's activation tail drains.
            # (Non-bank-aligned regions share a PSUM zero region, which
            # forbids concurrently open accumulation groups.)
            wfirst = (wc * 4) % 2048 == 0
            if wfirst:
                for r, wi, ti, bi, ui, di in gates:
                    nc.tensor.matmul(ps[:, r, :], wq[:, wi, :], xall[:, xs],
                                     start=True, stop=False)
            for r, wi, ti, bi, ui, di in gates:
                po = ps[:, r, :]
                if not wfirst:
                    nc.tensor.matmul(po, wq[:, wi, :], xall[:, xs],
                                     start=True, stop=False)
                nc.tensor.matmul(po, ubtp[:, ti, :], hm[:, 0, :],
                                 start=False, stop=False)
                nc.tensor.matmul(po, ubb[:, bi, :], hm[:, 1, :],
                                 start=False, stop=False)
                if ui is not None:
                    nc.tensor.matmul(po, uun[:, ui, :], hch[:, le],
                                     start=False, stop=False)
                nc.tensor.matmul(
                    po, bd[:, di, :], onesm[:, ms], start=False, stop=True
                )

            gs = work.tile([128, 2560], dt.float32, tag="gs", name="gs")
            gs = gs.rearrange("p (g n) -> p g n", n=wc)
            nc.scalar.activation(gs[:, 0:nsig, :], ps[:, 0:nsig, :], AF.Sigmoid)
            nc.scalar.activation(gs[:, iuu_, :], ps[:, iuu_, :], AF.Tanh)

            cs = c_t[l][:, ls]
            if top:
                nc.vector.tensor_mul(cs, gs[:, iu_, :], gs[:, iuu_, :])
            else:
                t1 = work.tile([128, 512], dt.float32, tag="t1", name="t1")[:, 0:wc]
                nc.vector.tensor_mul(t1, gs[:, 0, :], gs[:, 4, :])
                t2 = work.tile([128, 512], dt.float32, tag="t2", name="t2")[:, 0:wc]
                nc.gpsimd.tensor_mul(t2, gs[:, 1, :], cch[:, le])
                t3 = work.tile([128, 512], dt.float32, tag="t3", name="t3")[:, 0:wc]
                nc.vector.tensor_mul(t3, gs[:, 2, :], com)
                a1 = work.tile([128, 512], dt.float32, tag="a1", name="a1")[:, 0:wc]
                nc.gpsimd.tensor_add(a1, t1, t2)
                nc.vector.tensor_add(cs, a1, t3)

            tch = work.tile([128, 512], dt.float32, tag="tch", name="tch")[:, 0:wc]
            nc.scalar.activation(tch, cs, AF.Tanh)
            nc.vector.tensor_mul(h_t[l][:, ls], gs[:, io_, :], tch)

        def emit_warm():
            """Keep-warm filler so the PE p-state clock stays hot across the
            serial dependency tail of the small levels. Writes rotate over
            disjoint PSUM slices so the dummies pipeline with no WAW stalls."""
            wps = psL.tile([H, WC_LEAF], dt.float32, tag="psl", name="warm")
            for i in range(WARM_N):
                o = (i % 4) * 128
                nc.tensor.matmul(
                    wps[:, o : o + 128], wq[:, 0, :], xall[:, 512 : 512 + 128],
                    start=True, stop=True, skip_group_check=True,
                )

        # emission plan: leaf + L7 software-pipelined in pair order (L7
        # chunk k needs the leaf pair k), then the cascade: chunk sizes
        # halve down the tree and go in (j, Q/2+j) pair order so parent
        # chunks become ready while the child level is still draining.
        items = []
        for k in range(8):
            items += [("leaf", k), ("leaf", 8 + k), ("chunk", D - 1, 512 * k, 512)]
        for l in range(D - 2, -1, -1):
            if l <= 5 and WARM_N:
                items.append(("warm",))
            wc = min(WC, LVLN[l])
            Q = max(1, LVLN[l] // wc)
            order = [q for p in range(Q // 2) for q in (p, Q // 2 + p)] or [0]
            items += [("chunk", l, j * wc, wc) for j in order]

        # pipelined emission: each chunk's mask ops go on the DVE/Pool
        # queues one chunk BEFORE the previous chunk's compute tail, so the
        # PE's operands are ready at chunk boundaries. (Only for l >= 4 —
        # deeper levels' masks would order before a same-queue h write they
        # depend on.)
        prev = None
        for it in items:
            if it[0] == "chunk" and it[1] >= 4:
                _, l_, c0_, wc_ = it
                pre_ = emit_masks(l_, c0_, wc_)
                if prev:
                    prev()
                prev = (lambda l_=l_, c0_=c0_, wc_=wc_, pre_=pre_:
                        emit_chunk(l_, c0_, wc_, pre_))
            else:
                if prev:
                    prev()
                    prev = None
                if it[0] == "leaf":
                    emit_leaf(it[1])
                elif it[0] == "warm":
                    emit_warm()
                else:
                    emit_chunk(it[1], it[2], it[3])
        if prev:
            prev()

        nc.sync.dma_start(out=h_out_d[:, :], in_=h_t[0][:, :])
        nc.sync.dma_start(out=c_out_d[:, :], in_=c_t[0][:, :])

    nc.finalize()
    _CACHE["nc"] = nc
    return nc


def prep_shared_inputs(emb, W, bW, Ubin, bUbin, Uun, bUun):
    emb = np.asarray(emb, np.float32)
    W = np.asarray(W, np.float32)
    bW = np.asarray(bW, np.float32)
    Ubin = np.asarray(Ubin, np.float32)
    bUbin = np.asarray(bUbin, np.float32)
    Uun = np.asarray(Uun, np.float32)
    bUun = np.asarray(bUun, np.float32)

    ubt = Ubin[:, :H, :]  # [5, H, H] top half (left child)
    ubb_ = Ubin[:, H:, :]  # bottom half (right child)
    # Ubt' = Ubt - Uun for gates with a unary path (i, fl, o, u)
    ubtp = ubt.copy()
    for gi_, ui_ in ((0, 0), (1, 1), (3, 2), (4, 3)):
        ubtp[gi_] = ubt[gi_] - Uun[ui_]

    # bias rows [b_g; db_g] per gate (i, fl, fr, o, u)
    bcom = np.stack([
        bW[0] + bUun[0],
        bW[1] + bUun[1],
        bW[1] + bUbin[2],
        bW[2] + bUun[2],
        bW[3] + bUun[3],
    ])
    bdel = np.stack([
        bUbin[0] - bUun[0],
        bUbin[1] - bUun[1],
        np.zeros(H, np.float32),
        bUbin[3] - bUun[2],
        bUbin[4] - bUun[3],
    ])
    bd = np.stack([bcom, bdel]).astype(BF16)  # [2, 5, H]

    return dict(
        emb_bf=emb.astype(BF16),
        wq=np.ascontiguousarray(W.transpose(1, 0, 2)).astype(BF16),
        ubtp=np.ascontiguousarray(ubtp.transpose(1, 0, 2)).astype(BF16),
        ubb=np.ascontiguousarray(ubb_.transpose(1, 0, 2)).astype(BF16),
        uun=np.ascontiguousarray(Uun.transpose(1, 0, 2)).astype(BF16),
        bd=bd,
        bleaf=bW[3].reshape(H, 1).astype(np.float32),
    )


def prep_core_inputs(tokens_c, arity_c, shared):
    """Per-core inputs. tokens_c [BL,511], arity_c [BL,255]."""
    tokens_c = np.asarray(tokens_c)
    arity_c = np.asarray(arity_c)
    emb_bf = shared["emb_bf"]

    xcols = np.empty((N_ALL, E), dtype=BF16)
    mrow = np.empty(N_INT, dtype=np.float32)
    for l in [D] + list(range(D - 1, -1, -1)):
        off, cnt = 2 ** l - 1, 2 ** l
        toks = tokens_c[:, off : off + cnt].reshape(-1)[PERM[l]]
        xcols[XOFF[l] : XOFF[l] + LVLN[l]] = emb_bf[toks]
        if l < D:
            ar = arity_c[:, off : off + cnt].reshape(-1)[PERM[l]]
            mrow[MOFF[l] : MOFF[l] + LVLN[l]] = (ar == 1).astype(np.float32)

    m16 = mrow.astype(BF16)
    onesm = np.stack([np.ones(N_INT, BF16), m16])  # [2, N_INT]
    out = {k: v for k, v in shared.items() if k != "emb_bf"}
    out["xall"] = np.ascontiguousarray(xcols.T)
    out["mbc"] = np.broadcast_to(m16, (128, N_INT)).copy()
    out["onesm"] = onesm
    return out


def kernel(tokens, arity, emb, W, bW, Ubin, bUbin, Uun, bUun):
    from concourse.bass_utils import run_bass_kernel_spmd

    tokens = np.asarray(tokens)
    arity = np.asarray(arity)

    shared = prep_shared_inputs(emb, W, bW, Ubin, bUbin, Uun, bUun)
    in_maps = [
        prep_core_inputs(
            tokens[k * BL : (k + 1) * BL], arity[k * BL : (k + 1) * BL], shared
        )
        for k in range(NCORES)
    ]

    nc = _build_nc()
    res = run_bass_kernel_spmd(nc, in_maps, core_ids=list(range(NCORES)))
    results = res.results

    h = np.concatenate([r["h_out"].T for r in results], axis=0)
    c = np.concatenate([r["c_out"].T for r in results], axis=0)
    return h.astype(np.float32), c.astype(np.float32)
